# revision 14
# baseline (speedup 1.0000x reference)
"""Bounding-box kernel for Trainium2 (Bass/Tile), 8-core SPMD.

Problem: mask [128, 1, 512, 512] f32 -> bbox [128, 4] int32
  (y_min, x_min, y_max, x_max) of the region where mask >= 0.5,
  with (0, 0, H, W) when a row/col has no hit.

Strategy (per core, 16 images):
  - DMA each image [512, 512] as one [128, 4, 512] tile (H split into 4
    partition blocks).
  - Column extents: ACT computes t = Relu(x*2^24 + (1 - 2^23)) which is
    exactly 0 iff x < 0.5 and >= 1 otherwise (exact in f32 for the
    threshold boundary), output bf16. One-hot [128, 16] lhsT matmuls
    accumulate per-image column "counts" for all 16 images into a single
    PSUM [16, 512] tile (partition = image).
  - Row extents: Pool reduce_max over W per block -> [128, 64] (col =
    b*16 + i), compare >= 0.5, PE-transpose the four [128, 16] slices
    into PSUM [16, 512] (partition = image, free = H).
  - First/last hit index via fused DVE tensor_tensor_reduce:
      min(hit * (f - 512)) + 512  -> first hit  (512 if none)
      max(hit * (f + 1))          -> last hit+1 (0 if none)
    plus a no-hit fixup, f32 -> int32 copy, tiny DMA out.
"""

import numpy as np
import ml_dtypes
from contextlib import ExitStack

import concourse.bass as bass
import concourse.bacc as bacc
import concourse.tile as tile
import concourse.mybir as mybir
from concourse.bass_utils import run_bass_kernel_spmd

N_CORES = 8
N, H, W = 128, 512, 512
NPC = N // N_CORES          # images per core = 16
P = 128                     # SBUF partitions
NBLK = H // P               # 4 row blocks per image
F32 = mybir.dt.float32
BF16 = mybir.dt.bfloat16
I32 = mybir.dt.int32

# Relu(x * 2^25 - (2^24 - 1)) == 0 iff x < 0.5, >= 1 iff x >= 0.5, exact
# for EVERY f32 x: x*2^25 is exact (power-of-2 scale); for x < 0.5,
# x*2^25 <= 2^24 - 1 so the true sum is <= 0 (rounding is monotone, 0 is
# representable); for x >= 0.5 the true sum is >= 1 and rounds to >= 1.
ACT_SCALE = float(2**25)
ACT_BIAS = float(1 - 2**24)

TRACE = False               # test.py sets True to capture a HW profile
LAST_RESULTS = None         # BassKernelResults of the last run

_compiled = None


def _build_nc():
    nc = bacc.Bacc(
        "TRN2", target_bir_lowering=False, debug=False, num_devices=N_CORES
    )
    mask_d = nc.dram_tensor("mask", [NPC * H, W], F32, kind="ExternalInput").ap()
    oneh_d = nc.dram_tensor("onehot", [P, NPC * NPC], BF16, kind="ExternalInput").ap()
    ident_d = nc.dram_tensor("ident", [P, P], F32, kind="ExternalInput").ap()
    lo_d = nc.dram_tensor("lo_const", [NPC, W], F32, kind="ExternalInput").ap()
    hi_d = nc.dram_tensor("hi_const", [NPC, W], F32, kind="ExternalInput").ap()
    bbox_d = nc.dram_tensor("bbox", [NPC, 4], I32, kind="ExternalOutput").ap()

    with tile.TileContext(nc) as tc, ExitStack() as ctx:
        consts = ctx.enter_context(tc.tile_pool(name="consts", bufs=1))
        xpool = ctx.enter_context(tc.tile_pool(name="x", bufs=8))
        hpool = ctx.enter_context(tc.tile_pool(name="h", bufs=12))
        small = ctx.enter_context(tc.tile_pool(name="small", bufs=1))
        scratch = ctx.enter_context(tc.tile_pool(name="scratch", bufs=2))
        psum = ctx.enter_context(tc.tile_pool(name="psum", bufs=1, space="PSUM"))

        oneh = consts.tile([P, NPC * NPC], BF16)
        nc.sync.dma_start(out=oneh[:], in_=oneh_d)
        ident = consts.tile([P, P], F32)
        nc.sync.dma_start(out=ident[:], in_=ident_d)
        lo_c = consts.tile([NPC, W], F32)
        nc.sync.dma_start(out=lo_c[:], in_=lo_d)
        hi_c = consts.tile([NPC, W], F32)
        nc.sync.dma_start(out=hi_c[:], in_=hi_d)
        act_bias = consts.tile([P, 1], F32)
        nc.vector.memset(act_bias[:], ACT_BIAS)

        # col j = b*16 + i holds a "row hit-ness" value for image i, row
        # block b: for b in {0,1} it is the relu-mass row SUM (0 iff no
        # hit, else >= 1) from the ACT accumulator; for b in {2,3} it is
        # the raw row MAX from a DVE reduce. Either way hit <=> >= 0.5.
        rowmax = small.tile([P, NBLK * NPC], F32)
        rowmax_v = rowmax.rearrange("p (b i) -> p b i", b=NBLK)
        cnt_ps = psum.tile([NPC, W], F32)    # per-image column counts
        trow_ps = psum.tile([NPC, H], F32)   # per-image row hits (transposed)

        for i in range(NPC):
            x = xpool.tile([P, NBLK, W], F32)
            nc.sync.dma_start(
                out=x[:],
                in_=mask_d[i * H:(i + 1) * H, :].rearrange("(b p) w -> p b w", p=P),
            )
            h = hpool.tile([P, NBLK, W], BF16)
            for b in (0, 1):
                nc.scalar.activation(
                    h[:, b, :], x[:, b, :], mybir.ActivationFunctionType.Relu,
                    bias=act_bias[:], scale=ACT_SCALE,
                    accum_out=rowmax[:, b * NPC + i:b * NPC + i + 1],
                )
            nc.scalar.activation(
                h[:, 2:4, :], x[:, 2:4, :], mybir.ActivationFunctionType.Relu,
                bias=act_bias[:], scale=ACT_SCALE,
            )
            nc.vector.tensor_reduce(
                out=rowmax_v[:, 2:4, i], in_=x[:, 2:4, :],
                axis=mybir.AxisListType.X, op=mybir.AluOpType.max,
            )
            lhsT = oneh[:, i * NPC:(i + 1) * NPC]
            for b in range(NBLK):
                nc.tensor.matmul(
                    cnt_ps[:, :], lhsT, h[:, b, :],
                    start=(i == 0 and b == 0),
                    stop=(i == NPC - 1 and b == NBLK - 1),
                )

        # rows: hit01 then transpose blocks into [16, 512]
        rowhit = small.tile([P, NBLK * NPC], F32)
        nc.vector.tensor_scalar(
            rowhit[:], rowmax[:], 0.5, None, mybir.AluOpType.is_ge
        )
        rowhit_v = rowhit.rearrange("p (b i) -> p b i", b=NBLK)
        for b in range(NBLK):
            nc.tensor.matmul(
                trow_ps[:, b * P:(b + 1) * P], rowhit_v[:, b, :], ident[:],
                is_transpose=True, start=True, stop=True,
            )

        colhit = small.tile([NPC, W], F32)
        nc.vector.tensor_scalar(
            colhit[:], cnt_ps[:], 0.5, None, mybir.AluOpType.is_ge
        )

        # NOTE: tensor_tensor_reduce (custom DVE ISA op) crashes the exec
        # unit on this runtime path; use plain mul + reduce instead.
        def extents(hit_ap, lo_out, hi_out):
            prod = scratch.tile([NPC, W], F32, tag="prod")
            nc.vector.tensor_mul(prod[:], hit_ap, lo_c[:])
            nc.vector.tensor_reduce(
                out=lo_out, in_=prod[:],
                axis=mybir.AxisListType.X, op=mybir.AluOpType.min,
            )
            prod2 = scratch.tile([NPC, W], F32, tag="prod")
            nc.vector.tensor_mul(prod2[:], hit_ap, hi_c[:])
            nc.vector.tensor_reduce(
                out=hi_out, in_=prod2[:],
                axis=mybir.AxisListType.X, op=mybir.AluOpType.max,
            )

        ylo = small.tile([NPC, 1], F32)
        yhi = small.tile([NPC, 1], F32)
        xlo = small.tile([NPC, 1], F32)
        xhi = small.tile([NPC, 1], F32)
        extents(trow_ps[:], ylo[:], yhi[:])
        extents(colhit[:], xlo[:], xhi[:])

        # lo_final = (lo_raw + 512) * (1 - nohit); hi_final = hi_raw + 512*nohit
        # where nohit = (hi_raw == 0). bbox layout: (ymin, xmin, ymax, xmax).
        bbox_f = small.tile([NPC, 4], F32)

        def fixup(lo_raw, hi_raw, full, lo_col, hi_col):
            m = small.tile([NPC, 1], F32, tag="fix_m")
            nc.vector.tensor_scalar(m[:], hi_raw, 0.0, None, mybir.AluOpType.is_equal)
            t = small.tile([NPC, 1], F32, tag="fix_t")
            nc.vector.tensor_scalar_add(t[:], lo_raw, full)
            v = small.tile([NPC, 1], F32, tag="fix_v")
            nc.vector.tensor_mul(v[:], t[:], m[:])
            nc.vector.tensor_sub(bbox_f[:, lo_col:lo_col + 1], t[:], v[:])
            w_ = small.tile([NPC, 1], F32, tag="fix_w")
            nc.vector.tensor_scalar_mul(w_[:], m[:], full)
            nc.vector.tensor_add(bbox_f[:, hi_col:hi_col + 1], hi_raw, w_[:])

        fixup(ylo[:], yhi[:], float(H), 0, 2)
        fixup(xlo[:], xhi[:], float(W), 1, 3)

        bbox_i = small.tile([NPC, 4], I32)
        nc.vector.tensor_copy(bbox_i[:], bbox_f[:])
        nc.sync.dma_start(out=bbox_d, in_=bbox_i[:])

    nc.compile()
    return nc


def _consts():
    oneh = np.zeros((P, NPC * NPC), dtype=ml_dtypes.bfloat16)
    for i in range(NPC):
        oneh[:, i * NPC + i] = 1.0
    ident = np.eye(P, dtype=np.float32)
    f = np.arange(W, dtype=np.float32)
    lo = np.broadcast_to(f - W, (NPC, W)).copy()
    hi = np.broadcast_to(f + 1, (NPC, W)).copy()
    return oneh, ident, lo, hi


def kernel(mask):
    global _compiled, LAST_RESULTS
    mask = np.ascontiguousarray(np.asarray(mask), dtype=np.float32)
    assert mask.shape == (N, 1, H, W), mask.shape
    if _compiled is None:
        _compiled = _build_nc()
    nc = _compiled
    oneh, ident, lo, hi = _consts()
    m = mask.reshape(N, H, W)
    in_maps = []
    for c in range(N_CORES):
        in_maps.append({
            "mask": np.ascontiguousarray(
                m[c * NPC:(c + 1) * NPC].reshape(NPC * H, W)
            ),
            "onehot": oneh,
            "ident": ident,
            "lo_const": lo,
            "hi_const": hi,
        })
    res = run_bass_kernel_spmd(nc, in_maps, list(range(N_CORES)), trace=TRACE)
    LAST_RESULTS = res
    out = np.concatenate([res.results[c]["bbox"] for c in range(N_CORES)], axis=0)
    return out.astype(np.int32, copy=False)
